# revision 26
# baseline (speedup 1.0000x reference)
"""ExllamaV3 trellis-dequant linear layer on 8 Trainium2 NeuronCores — v4.

y = x @ W,  W = diag(suh) . H128 . dequant(trellis) . H128 . diag(svh)

Column-parallel over out_features (512 cols/core). Per core:
  Phase W (~60us): host-unpacked (vh, vl) u8 byte streams -> split-precision
    LCG chain spread over Scalar (t1), GpSimd (t3, t4) and DVE (slo,
    shi-with-fused-carry, rlo, rhi) -> wi f16 -> hL matmul (PE) + suh row
    scale -> W2[kb] f16 [128k, 512n]. The consumer (hL matmuls, W2 scale,
    trailing) runs one batch behind the producer so no engine queue ever
    head-blocks on the cross-engine chain.
  Trailing: slab0 nb0-3 + slab1 nb0-2 main matmuls emitted kb-outer
    (7 open psum groups) so the PE consumes each W2[kb] the moment it lands.
  Steady (~180us): per remaining (slab, out-block): accumulate 32 f16
    matmuls with out = [n-part, t-free], drain psum -> ysb f16, hR matmul
    in f16 (+-1 exact), svh scale, DMA y shard n-major.
"""
import numpy as np
from contextlib import ExitStack

import concourse.bass as bass  # noqa: F401  (registers AP/engine types)
import concourse.tile as tile
from concourse import bacc, mybir
from concourse import bass_utils

Alu = mybir.AluOpType
Act = mybir.ActivationFunctionType
f32 = mybir.dt.float32
f16 = mybir.dt.float16
i32 = mybir.dt.int32
u16 = mybir.dt.uint16
u8 = mybir.dt.uint8

TOKENS = 4096
IN_F = 4096
OUT_F = 4096
NCORES = 8
NSH = OUT_F // NCORES          # 512 out cols per core
Kt = IN_F // 16                # 256
NTS = (OUT_F // 16) // NCORES  # 32 trellis tile-cols per core
NKB = IN_F // 128              # 32 contraction chunks
NTQ = TOKENS // 512            # 8 token slabs
SPP = NKB * NTS * 2 * 8        # 16384 stream elems per partition

MULT = 89226354
ADD = 64248484
C2h, C2l = MULT >> 16, MULT & 0xFFFF
_C1 = (MULT * 256) & 0xFFFFFFFF
C1h, C1l = _C1 >> 16, _C1 & 0xFFFF
ADDh, ADDl = ADD >> 16, ADD & 0xFFFF

DEQ_FD = 1024                  # dequant batch free-dim (2 k-chunks)
NBATCH = SPP // DEQ_FD         # 16

# ---------------- host-side layout helpers (pure layout, no math) ----------

_p = np.arange(128)
_a_p = _p // 16                 # kt % 8
_r_p = _p % 16                  # k % 16
_c_p = np.where(_r_p < 8, _r_p % 2, 2 + (_r_p % 2))
_tr_p = np.where(_r_p < 8, _r_p // 2, (_r_p - 8) // 2)
SH_L = (12 - 4 * _c_p).astype(np.int64)   # per-partition 16-bit window shift

_s = np.arange(SPP)
_tc_s = _s % 8
_jh_s = (_s // 8) % 2
_nt_s = (_s // 16) % NTS
_kb_s = _s // (16 * NTS)

_KT_IDX = 8 * _kb_s[None, :] + _a_p[:, None]
_J_IDX = 8 * _tc_s[None, :] + 2 * _tr_p[:, None] + _jh_s[None, :]
_JM1_IDX = (_J_IDX - 1) % 64
_NT_IDX = np.broadcast_to(_nt_s[None, :], (128, SPP))


def _hadamard128():
    h = np.array([[1]], dtype=np.int64)
    while h.shape[0] < 128:
        h = np.block([[h, h], [h, -h]])
    return h.astype(np.float32)


def _pack_bytes(w16_core):
    """[Kt, NTS, 64] uint16 (pair-swapped) -> (vh, vl) uint8 [128, SPP].

    Pure bitfield repacking: the 16-bit encoded window at each stream
    position, split into its high/low bytes.
    """
    w0 = w16_core[_KT_IDX, _NT_IDX, _J_IDX].astype(np.uint32)
    w1 = w16_core[_KT_IDX, _NT_IDX, _JM1_IDX].astype(np.uint32)
    V = (w1 << 16) | w0
    v16 = (V >> SH_L[:, None]) & 0xFFFF
    return (v16 >> 8).astype(np.uint8), (v16 & 0xFF).astype(np.uint8)


# ---------------- device program ------------------------------------------

def _build_program():
    nc = bacc.Bacc(
        "TRN2",
        target_bir_lowering=False,
        debug=False,
        enable_asserts=False,
        num_devices=NCORES,
    )

    # xT_in[tq, p, kb, t] = x[tq*512 + t, kb*128 + p]  (f16)
    xT_d = nc.dram_tensor("xT_in", [NTQ, 128, NKB * 512], f16, kind="ExternalInput")
    vh_d = nc.dram_tensor("vh_in", [128, SPP], u8, kind="ExternalInput")
    vl_d = nc.dram_tensor("vl_in", [128, SPP], u8, kind="ExternalInput")
    suh_d = nc.dram_tensor("suh_in", [128, NKB], f32, kind="ExternalInput")
    svh_d = nc.dram_tensor("svh_in", [128, 4], f32, kind="ExternalInput")
    y_d = nc.dram_tensor("y_out", [NSH, TOKENS], f32, kind="ExternalOutput")

    had = _hadamard128()
    hL_d = nc.inline_tensor((had / 128.0).astype(np.float16), name="hL")
    hR_d = nc.inline_tensor(had.astype(np.float16), name="hR")

    with tile.TileContext(nc) as tc, ExitStack() as ctx:
        cpool = ctx.enter_context(tc.tile_pool(name="consts", bufs=1))
        hL = cpool.tile([128, 128], f16)
        hR = cpool.tile([128, 128], f16)
        suh = cpool.tile([128, NKB], f32)
        svh = cpool.tile([128, 4], f32)
        nc.sync.dma_start(hL[:], hL_d.ap())
        nc.sync.dma_start(hR[:], hR_d.ap())
        nc.sync.dma_start(suh[:], suh_d.ap())
        nc.sync.dma_start(svh[:], svh_d.ap())

        w2pool = ctx.enter_context(tc.tile_pool(name="w2", bufs=NKB))
        W2 = [w2pool.tile([128, NSH], f16, tag="w2", name=f"w2_{i}")
              for i in range(NKB)]

        xq_pool = ctx.enter_context(tc.tile_pool(name="xq", bufs=3))
        vs_pool = ctx.enter_context(tc.tile_pool(name="vsin", bufs=2))
        dq = ctx.enter_context(tc.tile_pool(name="deq", bufs=2))
        ysb_pool = ctx.enter_context(tc.tile_pool(name="ysb", bufs=9))
        zsb_pool = ctx.enter_context(tc.tile_pool(name="zsb", bufs=6))

        psw_pool = ctx.enter_context(tc.tile_pool(name="psw", bufs=1, space="PSUM"))
        trail_pool = ctx.enter_context(tc.tile_pool(name="trail", bufs=4, space="PSUM"))
        acc_pool = ctx.enter_context(tc.tile_pool(name="acc", bufs=3, space="PSUM"))

        xq = [None] * NTQ

        def load_slab(tq):
            t = xq_pool.tile([128, NKB * 512], f16, tag="xq", name=f"xq_{tq}")
            nc.sync.dma_start(t[:], xT_d.ap()[tq])
            xq[tq] = t

        # slabs 0/1 arrive as quarters interleaved with the dequant batches so
        # the byte streams are never queued behind megabytes of x
        xq[0] = xq_pool.tile([128, NKB * 512], f16, tag="xq", name="xq_0")
        xq[1] = xq_pool.tile([128, NKB * 512], f16, tag="xq", name="xq_1")

        # ---- Phase W: dequant + left Hadamard + suh, trailing slabs 0/1 ----
        trail = [trail_pool.tile([128, 512], f32, tag="trail", name=f"trail_{nb}")
                 for nb in range(4)]
        trail2 = [acc_pool.tile([128, 512], f32, tag="acc", name=f"trail2_{nb}")
                  for nb in range(3)]

        pipe = [None] * NBATCH  # software pipeline: (rlo, rhi) per batch
        vs = [None] * NBATCH    # (VH, VL) per batch

        def emit_dma(b):
            sl = slice(b * DEQ_FD, (b + 1) * DEQ_FD)
            VH = vs_pool.tile([128, DEQ_FD], u8, tag="vh")
            VL = vs_pool.tile([128, DEQ_FD], u8, tag="vl")
            nc.sync.dma_start(VH[:], vh_d.ap()[:, sl])
            nc.sync.dma_start(VL[:], vl_d.ap()[:, sl])
            vs[b] = (VH, VL)
            if b < 8:
                # slabs 0/1 stream in as eighths over the early batches so the
                # byte-stream DMAs are never queued behind a burst of x
                qs = slice(b * 4 * 512, (b + 1) * 4 * 512)
                nc.sync.dma_start(xq[0][:, qs], xT_d.ap()[0][:, qs])
                nc.sync.dma_start(xq[1][:, qs], xT_d.ap()[1][:, qs])
            else:
                # slab 2 prefetch: its tile is free (xq bufs=3) and the DMA
                # queues are idle once slabs 0/1 have landed; sixteenths keep
                # each intrusion ahead of the next batch's byte streams small
                if xq[2] is None:
                    xq[2] = xq_pool.tile([128, NKB * 512], f16, tag="xq",
                                         name="xq_2")
                for h in range(2):
                    qs = slice(((b - 8) * 2 + h) * 2 * 512,
                               ((b - 8) * 2 + h + 1) * 2 * 512)
                    nc.sync.dma_start(xq[2][:, qs], xT_d.ap()[2][:, qs])

        def emit_t1(b):
            VH, VL = vs[b]
            t1 = dq.tile([128, DEQ_FD], i32, tag="t1")
            nc.scalar.activation(t1[:], VL[:], Act.Copy, bias=float(ADDl), scale=float(C2l))
            return t1

        def emit_t3(b):
            VH, VL = vs[b]
            t3 = dq.tile([128, DEQ_FD], i32, tag="t3")
            nc.scalar.activation(t3[:], VL[:], Act.Copy, bias=float(ADDh), scale=float(C2h))
            return t3

        def emit_dve(b, t1, t3):
            VH, VL = vs[b]
            # DVE: slo = vh*C1l + t1 ; rlo = (slo & 0x8FFF) ^ 0x3B60
            #      t4 = vh*C1h + t3 ; shi = (slo >> 16) + (t4 & 0xFFFF) —
            #      the shift/mask are free u16 views of the i32 tensors
            slo = dq.tile([128, DEQ_FD], i32, tag="slo")
            t4 = dq.tile([128, DEQ_FD], i32, tag="t4")
            shi = dq.tile([128, DEQ_FD], i32, tag="shi")
            rlo = dq.tile([128, DEQ_FD], u16, tag="rlo")
            rhi = dq.tile([128, DEQ_FD], u16, tag="rhi")
            nc.vector.scalar_tensor_tensor(slo[:], VH[:], C1l, t1[:], Alu.mult, Alu.add)
            slo_h = slo[:].bitcast(u16).rearrange("p (f two) -> p f two", two=2)
            nc.vector.tensor_scalar(rlo[:], slo_h[:, :, 0], 0x8FFF, 0x3B60,
                                    Alu.bitwise_and, Alu.bitwise_xor)
            nc.vector.scalar_tensor_tensor(t4[:], VH[:], C1h, t3[:], Alu.mult, Alu.add)
            t4_h = t4[:].bitcast(u16).rearrange("p (f two) -> p f two", two=2)
            nc.vector.tensor_tensor(shi[:], slo_h[:, :, 1], t4_h[:, :, 0], Alu.add)
            shi_h = shi[:].bitcast(u16).rearrange("p (f two) -> p f two", two=2)
            nc.vector.tensor_scalar(rhi[:], shi_h[:, :, 0], 0x8FFF, 0x3B60,
                                    Alu.bitwise_and, Alu.bitwise_xor)
            pipe[b] = (rlo, rhi)

        def emit_trailing(kb):
            # slabs 0/1: 7 open psum groups, consumed the moment W2[kb] lands
            for nb in range(4):
                nc.tensor.matmul(trail[nb][:],
                                 W2[kb][:, nb * 128:(nb + 1) * 128],
                                 xq[0][:, kb * 512:(kb + 1) * 512],
                                 start=(kb == 0), stop=(kb == NKB - 1),
                                 skip_group_check=True)
            for nb in range(3):
                nc.tensor.matmul(trail2[nb][:],
                                 W2[kb][:, nb * 128:(nb + 1) * 128],
                                 xq[1][:, kb * 512:(kb + 1) * 512],
                                 start=(kb == 0), stop=(kb == NKB - 1),
                                 skip_group_check=True)

        def emit_hl(b, q):
            rlo, rhi = pipe[b]
            lo16 = rlo[:].bitcast(f16)
            hi16 = rhi[:].bitcast(f16)
            kb = 2 * b + q
            psw = psw_pool.tile([128, 512], f32)
            # W_inner = f16(lo) + f16(hi) folded into the accumulation:
            # H(a+b) = Ha + Hb, both moving operands contiguous f16
            nc.tensor.matmul(psw[:], hL[:], lo16[:, q * 512:(q + 1) * 512],
                             start=True, stop=False, skip_group_check=True)
            nc.tensor.matmul(psw[:], hL[:], hi16[:, q * 512:(q + 1) * 512],
                             start=False, stop=True, skip_group_check=True)
            nc.scalar.mul(W2[kb][:], psw[:], suh[:, kb:kb + 1])

        # Emission order is engineered per engine queue:
        #  scalar: [t1(b), W2mul(2b-4), t3(b), W2mul(2b-3)] — t1/t3 stay a
        #    full batch ahead of the DVE, W2 psum drains land between them
        #    so neither the DVE chain nor the PE's hL WAR ever head-block.
        #  PE: [hLq0(b-2), trail(2b-5), hLq1(b-2), trail(2b-4)] — trailing
        #    lags one k-chunk, covering the single-psw-bank WAR window.
        emit_dma(0)
        emit_dma(1)
        t1b, t3b = emit_t1(0), emit_t3(0)
        emit_dve(0, t1b, t3b)
        for b in range(1, NBATCH):
            emit_dma(b + 1) if b + 1 < NBATCH else None
            t1b = emit_t1(b)
            if b >= 2:
                emit_hl(b - 2, 0)
                if 2 * (b - 2) >= 1:
                    emit_trailing(2 * (b - 2) - 1)
            t3b = emit_t3(b)
            if b >= 2:
                emit_hl(b - 2, 1)
                emit_trailing(2 * (b - 2))
            emit_dve(b, t1b, t3b)
        for b2 in (NBATCH - 2, NBATCH - 1):
            emit_hl(b2, 0)
            emit_trailing(2 * b2 - 1)
            emit_hl(b2, 1)
            emit_trailing(2 * b2)
        emit_trailing(NKB - 1)

        # ---- Steady slabs + pipelined drain (hR f16 + svh + DMA) ----
        pending = []  # (ysb, tq, nb)

        def flush_one():
            ysb, tq, nb = pending.pop(0)
            # psz rotates through the trailing pool, whose banks are free
            # after the trailing drains — keeps acc_pool solely for the
            # main accumulation groups (no WAR stall at group boundaries)
            psz = trail_pool.tile([128, 512], f32, tag="trail", name=f"psz_{tq}_{nb}")
            nc.tensor.matmul(psz[:], hR[:], ysb[:], start=True, stop=True)
            zsb = zsb_pool.tile([128, 512], f32, tag="zsb")
            nc.scalar.mul(zsb[:], psz[:], svh[:, nb:nb + 1])
            nc.sync.dma_start(
                y_d.ap()[nb * 128:(nb + 1) * 128, tq * 512:(tq + 1) * 512], zsb[:])

        def drain(acc, tq, nb):
            ysb = ysb_pool.tile([128, 512], f16, tag="ysb")
            nc.scalar.copy(ysb[:], acc[:])
            pending.append((ysb, tq, nb))

        for nb in range(4):
            drain(trail[nb], 0, nb)
        for nb in range(3):
            drain(trail2[nb], 1, nb)

        for tq in range(1, NTQ):
            if 3 <= tq + 1 < NTQ:
                load_slab(tq + 1)
            for nb in range(4):
                if tq == 1 and nb < 3:
                    continue
                acc = acc_pool.tile([128, 512], f32, tag="acc", name=f"acc_{tq}_{nb}")
                for kb in range(NKB):
                    nc.tensor.matmul(acc[:],
                                     W2[kb][:, nb * 128:(nb + 1) * 128],
                                     xq[tq][:, kb * 512:(kb + 1) * 512],
                                     start=(kb == 0), stop=(kb == NKB - 1))
                drain(acc, tq, nb)
                while len(pending) > 1:
                    flush_one()
        while pending:
            flush_one()

    nc.compile()
    return nc


_NC_CACHE = None
LAST_RESULT = None


def _get_program():
    global _NC_CACHE
    if _NC_CACHE is None:
        _NC_CACHE = _build_program()
    return _NC_CACHE


def kernel(x, trellis, suh, svh):
    global LAST_RESULT
    x = np.asarray(x, dtype=np.float32)
    trellis = np.asarray(trellis)
    suh = np.asarray(suh, dtype=np.float32)
    svh = np.asarray(svh, dtype=np.float32)

    # host layout prep (pure re-layout / bitfield repacking, no arithmetic)
    w16 = (trellis.astype(np.uint32) & 0xFFFF).astype(np.uint16)
    w16 = w16.reshape(Kt, OUT_F // 16, 32, 2)[..., ::-1].reshape(Kt, OUT_F // 16, 64)
    suh_r = np.ascontiguousarray(suh.reshape(NKB, 128).T)
    # xT[tq, p, kb, t] = x[tq*512+t, kb*128+p]
    xT = np.ascontiguousarray(
        x.reshape(NTQ, 512, NKB, 128).transpose(0, 3, 2, 1)
    ).astype(np.float16).reshape(NTQ, 128, NKB * 512)

    in_maps = []
    for c in range(NCORES):
        w16c = w16[:, c * NTS:(c + 1) * NTS, :]
        vh, vl = _pack_bytes(w16c)
        svh_r = np.ascontiguousarray(svh[c * NSH:(c + 1) * NSH].reshape(4, 128).T)
        in_maps.append({
            "xT_in": xT,
            "vh_in": vh,
            "vl_in": vl,
            "suh_in": suh_r,
            "svh_in": svh_r,
        })

    nc = _get_program()
    res = bass_utils.run_bass_kernel_spmd(nc, in_maps, core_ids=list(range(NCORES)))
    LAST_RESULT = res

    y = np.empty((TOKENS, OUT_F), dtype=np.float32)
    for c in range(NCORES):
        y[:, c * NSH:(c + 1) * NSH] = res.results[c]["y_out"].T
    return y
